# revision 4
# baseline (speedup 1.0000x reference)
"""Trainium2 Bass kernel for DigitConvolutionalModel.

Reference computation (B = 32768):
    x: [B, 784] -> reshape [B, 28, 28]
    conv 3x3 valid with w_conv -> [B, 26, 26] -> [B, 676]
    h1 = relu(conv @ W1 + b1)    W1: [676, 100]
    h2 = relu(h1 @ W2 + b2)      W2: [100, 100]
    out = h2 @ W3 + b3           W3: [100, 10]

Strategy
--------
Pure data parallel: batch is split 8 ways (4096 rows/core); weights are
replicated. The conv is a linear map, so it is folded into W1 on the host:
    conv(x) @ W1 == x @ (M @ W1) = x @ W1e,  W1e: [784, 100]
where M[p, q] scatters w_conv over the valid-conv banded structure. This
is exact up to fp rounding and removes the conv from the device entirely.

On-device layout is "transposed": features on SBUF partitions, batch on the
free dimension, so each layer's matmul output in PSUM feeds the next layer
directly as the moving operand (contraction dim = partitions). The host
pre-transposes x per core to [784, 4096] and blocks it as
[n_tiles, 112, 7, NT] so every DMA descriptor is a long contiguous run.

Per batch tile of NT columns:
    ps1[100, NT]  = sum_c W1e[c*112:(c+1)*112, :100].T @ xT[c*112.., NT]
    h1 = relu(ps1 + b1)         (scalar engine, bias per partition)
    ps2 = W2.T @ h1 ; h2 = relu(ps2 + b2)   (vector engine)
    ps3 = W3.T @ h2 ; out = ps3 + b3
Matmuls run as float32r (full PE rate at N>=256); the kernel is
HBM-bandwidth bound on streaming x.
"""

import numpy as np

N_CORES = 8
B = 32768
B_LOC = B // N_CORES          # 4096 rows per core
NT = 512                      # batch columns per tile (fp32 matmul max free dim)
N_TILES = B_LOC // NT         # 8
P = 112                       # partitions per K-chunk (784 = 7 * 112)
KC = 7                        # K chunks
H = 100                       # hidden width
O = 10                        # output width

_COMPILED = {}
LAST_RESULTS = None


def _build_nc(mm_dtype_name="float32r"):
    import concourse.mybir as mybir
    from concourse import bacc
    from concourse.tile import TileContext

    f32 = mybir.dt.float32
    fr = getattr(mybir.dt, mm_dtype_name)

    nc = bacc.Bacc(
        "TRN2", target_bir_lowering=False, debug=False, num_devices=N_CORES
    )
    xt = nc.dram_tensor("xt", [N_TILES, P, KC, NT], fr, kind="ExternalInput")
    w1 = nc.dram_tensor("w1", [P, KC, H], fr, kind="ExternalInput")
    w2 = nc.dram_tensor("w2", [H, H], fr, kind="ExternalInput")
    w3 = nc.dram_tensor("w3", [H, O], fr, kind="ExternalInput")
    b1 = nc.dram_tensor("b1", [H, 1], f32, kind="ExternalInput")
    b2 = nc.dram_tensor("b2", [H, 1], f32, kind="ExternalInput")
    b3 = nc.dram_tensor("b3", [O, 1], f32, kind="ExternalInput")
    ot = nc.dram_tensor("ot", [O, B_LOC], f32, kind="ExternalOutput")

    relu = mybir.ActivationFunctionType.Relu
    add = mybir.AluOpType.add
    amax = mybir.AluOpType.max

    with TileContext(nc) as tc:
        with (
            tc.tile_pool(name="wpool", bufs=1) as wpool,
            tc.tile_pool(name="xpool", bufs=3) as xpool,
            tc.tile_pool(name="hpool", bufs=2) as hpool,
            tc.tile_pool(name="opool", bufs=2) as opool,
            tc.tile_pool(name="ppool", bufs=2, space="PSUM") as ppool,
        ):
            w1_t = wpool.tile([P, KC, H], fr)
            nc.sync.dma_start(out=w1_t, in_=w1.ap())
            w2_t = wpool.tile([H, H], fr)
            nc.sync.dma_start(out=w2_t, in_=w2.ap())
            w3_t = wpool.tile([H, O], fr)
            nc.sync.dma_start(out=w3_t, in_=w3.ap())
            b1_t = wpool.tile([H, 1], f32)
            nc.sync.dma_start(out=b1_t, in_=b1.ap())
            b2_t = wpool.tile([H, 1], f32)
            nc.sync.dma_start(out=b2_t, in_=b2.ap())
            b3_t = wpool.tile([O, 1], f32)
            nc.sync.dma_start(out=b3_t, in_=b3.ap())

            for t in range(N_TILES):
                x_t = xpool.tile([P, KC, NT], fr)
                nc.sync.dma_start(out=x_t, in_=xt.ap()[t])

                ps1 = ppool.tile([128, NT], f32, tag="ps1")
                for c in range(KC):
                    nc.tensor.matmul(
                        ps1[:H, :],
                        lhsT=w1_t[:, c, :],
                        rhs=x_t[:, c, :],
                        start=(c == 0),
                        stop=(c == KC - 1),
                    )
                h1 = hpool.tile([H, NT], fr, tag="h1")
                nc.scalar.activation(h1, ps1[:H, :], relu, bias=b1_t)

                ps2 = ppool.tile([128, NT], f32, tag="ps2")
                nc.tensor.matmul(
                    ps2[:H, :], lhsT=w2_t, rhs=h1, start=True, stop=True
                )
                h2 = hpool.tile([H, NT], fr, tag="h2")
                nc.vector.tensor_scalar(h2, ps2[:H, :], b2_t, 0.0, add, amax)

                ps3 = ppool.tile([128, NT], f32, tag="ps3")
                nc.tensor.matmul(
                    ps3[:O, :], lhsT=w3_t, rhs=h2, start=True, stop=True
                )
                o_t = opool.tile([O, NT], f32)
                nc.vector.tensor_scalar_add(o_t, ps3[:O, :], b3_t)
                nc.sync.dma_start(out=ot.ap()[:, t * NT : (t + 1) * NT], in_=o_t)

    nc.finalize()
    return nc


def _fold_conv_into_w1(w_conv, W1):
    """W1e[784, 100] such that x @ W1e == conv3x3(x) @ W1 (exact linear fold)."""
    W1e = np.zeros((28, 28, H), np.float64)
    W1r = W1.astype(np.float64).reshape(26, 26, H)
    wc = w_conv.astype(np.float64)
    for di in range(3):
        for dj in range(3):
            W1e[di : di + 26, dj : dj + 26, :] += wc[di, dj] * W1r
    return W1e.reshape(784, H).astype(np.float32)


def kernel(x, w_conv, W1, b1, W2, b2, W3, b3):
    from concourse.bass_utils import run_bass_kernel_spmd

    global LAST_RESULTS

    x = np.ascontiguousarray(np.asarray(x, np.float32))
    W1e = _fold_conv_into_w1(np.asarray(w_conv), np.asarray(W1))
    # [784, 100] -> [P, KC, 100] with pixel q = c*112 + p at [p, c, :]
    w1_dev = np.ascontiguousarray(W1e.reshape(KC, P, H).transpose(1, 0, 2))
    w2_dev = np.ascontiguousarray(np.asarray(W2, np.float32))
    w3_dev = np.ascontiguousarray(np.asarray(W3, np.float32))
    b1_dev = np.asarray(b1, np.float32).reshape(H, 1).copy()
    b2_dev = np.asarray(b2, np.float32).reshape(H, 1).copy()
    b3_dev = np.asarray(b3, np.float32).reshape(O, 1).copy()

    in_maps = []
    for c in range(N_CORES):
        xs = x[c * B_LOC : (c + 1) * B_LOC]          # [B_LOC, 784]
        # -> [784, B_LOC] -> [N_TILES, P, KC, NT]: element [t, p, k, n]
        #    = x[t*NT + n, k*112 + p]
        xt = np.ascontiguousarray(
            xs.T.reshape(KC, P, N_TILES, NT).transpose(2, 1, 0, 3)
        )
        in_maps.append(
            {
                "xt": xt,
                "w1": w1_dev,
                "w2": w2_dev,
                "w3": w3_dev,
                "b1": b1_dev,
                "b2": b2_dev,
                "b3": b3_dev,
            }
        )

    key = "float32r"
    if key not in _COMPILED:
        _COMPILED[key] = _build_nc(key)
    nc = _COMPILED[key]

    res = run_bass_kernel_spmd(nc, in_maps, core_ids=list(range(N_CORES)))
    LAST_RESULTS = res

    out = np.empty((B, O), np.float32)
    for c in range(N_CORES):
        out[c * B_LOC : (c + 1) * B_LOC] = res.results[c]["ot"].T
    return out


# revision 9
# speedup vs baseline: 1.2222x; 1.2222x over previous
"""Trainium2 Bass kernel for DigitConvolutionalModel.

Reference computation (B = 32768):
    x: [B, 784] -> reshape [B, 28, 28]
    conv 3x3 valid with w_conv -> [B, 26, 26] -> [B, 676]
    h1 = relu(conv @ W1 + b1)    W1: [676, 100]
    h2 = relu(h1 @ W2 + b2)      W2: [100, 100]
    out = h2 @ W3 + b3           W3: [100, 10]

Strategy
--------
Pure data parallel: batch split 8 ways (4096 rows/core), weights replicated.
The conv is linear, so it is folded into W1 on the host:
    conv(x) @ W1 == x @ (M @ W1) = x @ W1e,  W1e: [784, 100]
removing the conv from the device entirely (exact up to fp rounding).

On-device layout is "transposed": features on SBUF partitions, batch on the
free dimension, so each layer's PSUM output feeds the next matmul directly
as the moving operand. The host pre-transposes x per core and blocks it as
[n_dma_tiles, 128, KC, NTD] (contraction split 784 = 6*128 + 16; the 16-row
tail is loaded once as a [16, B_LOC] resident tile) so every x DMA uses all
128 partitions with long contiguous runs.

Matmuls run as float32r (TF32-like single-pass mode: ~280 ns per N=512
matmul warm). The kernel is HBM-bandwidth bound streaming x (~12.8 MB/core).
"""

import numpy as np

N_CORES = 8
B = 32768
B_LOC = B // N_CORES          # 4096 rows per core
NT = 512                      # matmul moving-dim tile (fp32 max free dim)
NTD = 1024                    # batch columns per x DMA tile
N_DMA = B_LOC // NTD          # 4
HALves = NTD // NT            # 2
KC = 6                        # full 128-row contraction chunks
KT = 784 - KC * 128           # 16-row tail
H = 100                       # hidden width
O = 10                        # output width
XBUFS = 3                     # x tile double-buffer depth

_COMPILED = {}
LAST_RESULTS = None


def _build_nc():
    import concourse.mybir as mybir
    from concourse import bacc
    from concourse.tile import TileContext

    f32 = mybir.dt.float32
    fr = mybir.dt.float32r

    nc = bacc.Bacc(
        "TRN2", target_bir_lowering=False, debug=False, num_devices=N_CORES
    )
    xt = nc.dram_tensor("xt", [N_DMA, 128, KC, NTD], fr, kind="ExternalInput")
    xtl = nc.dram_tensor("xtl", [KT, B_LOC], fr, kind="ExternalInput")
    w1 = nc.dram_tensor("w1", [128, KC, H], fr, kind="ExternalInput")
    w1l = nc.dram_tensor("w1l", [KT, H], fr, kind="ExternalInput")
    # packed [100, 110]: W2 | W3
    w23 = nc.dram_tensor("w23", [H, H + O], fr, kind="ExternalInput")
    # packed [100, 3]: b1 | b2 | b3 (b3 on partitions 0..9)
    bb = nc.dram_tensor("bb", [H, 3], f32, kind="ExternalInput")
    ot = nc.dram_tensor("ot", [O, B_LOC], f32, kind="ExternalOutput")

    relu = mybir.ActivationFunctionType.Relu
    add = mybir.AluOpType.add
    amax = mybir.AluOpType.max

    with TileContext(nc) as tc:
        with (
            tc.tile_pool(name="wpool", bufs=1) as wpool,
            tc.tile_pool(name="xpool", bufs=XBUFS) as xpool,
            tc.tile_pool(name="hpool", bufs=3) as hpool,
            tc.tile_pool(name="opool", bufs=3) as opool,
            tc.tile_pool(name="ppool", bufs=2, space="PSUM") as ppool,
        ):
            # x tile 0 goes first on the sync HWDGE ring; weights go on the
            # scalar HWDGE ring so they don't delay the x stream.
            x_tiles = []
            x_t = xpool.tile([128, KC, NTD], fr, tag="xt")
            nc.sync.dma_start(out=x_t, in_=xt.ap()[0])
            x_tiles.append(x_t)

            w1_t = wpool.tile([128, KC, H], fr)
            nc.scalar.dma_start(out=w1_t, in_=w1.ap())
            w1l_t = wpool.tile([KT, H], fr)
            nc.scalar.dma_start(out=w1l_t, in_=w1l.ap())
            w23_t = wpool.tile([H, H + O], fr)
            nc.scalar.dma_start(out=w23_t, in_=w23.ap())
            bb_t = wpool.tile([H, 3], f32)
            nc.scalar.dma_start(out=bb_t, in_=bb.ap())
            xl_t = wpool.tile([KT, B_LOC], fr)
            nc.scalar.dma_start(out=xl_t, in_=xtl.ap())

            w2_t = w23_t[:, 0:H]
            w3_t = w23_t[:, H : H + O]
            b1_t = bb_t[:, 0:1]
            b2_t = bb_t[:, 1:2]
            b3_t = bb_t[:O, 2:3]

            for t in range(N_DMA):
                if t + 1 < N_DMA and len(x_tiles) <= t + 1:
                    x_n = xpool.tile([128, KC, NTD], fr, tag="xt")
                    nc.sync.dma_start(out=x_n, in_=xt.ap()[t + 1])
                    x_tiles.append(x_n)
                x_t = x_tiles[t]
                for hf in range(HALves):
                    n0 = hf * NT
                    gb = t * NTD + n0  # global batch offset within core
                    ps1 = ppool.tile([128, NT], f32, tag="ps1", bufs=3)
                    for c in range(KC):
                        nc.tensor.matmul(
                            ps1[:H, :],
                            lhsT=w1_t[:, c, :],
                            rhs=x_t[:, c, n0 : n0 + NT],
                            start=(c == 0),
                            stop=False,
                        )
                    nc.tensor.matmul(
                        ps1[:H, :],
                        lhsT=w1l_t,
                        rhs=xl_t[:, gb : gb + NT],
                        start=False,
                        stop=True,
                    )
                    h1 = hpool.tile([H, NT], fr, tag="h1")
                    nc.scalar.activation(h1, ps1[:H, :], relu, bias=b1_t)

                    ps2 = ppool.tile([128, NT], f32, tag="ps2", bufs=2)
                    nc.tensor.matmul(
                        ps2[:H, :], lhsT=w2_t, rhs=h1, start=True, stop=True
                    )
                    h2 = hpool.tile([H, NT], fr, tag="h2")
                    nc.vector.tensor_scalar(h2, ps2[:H, :], b2_t, 0.0, add, amax)

                    ps3 = ppool.tile([128, NT], f32, tag="ps3", bufs=2)
                    nc.tensor.matmul(
                        ps3[:O, :], lhsT=w3_t, rhs=h2, start=True, stop=True
                    )
                    o_t = opool.tile([O, NT], f32)
                    nc.vector.tensor_scalar_add(o_t, ps3[:O, :], b3_t)
                    nc.scalar.dma_start(
                        out=ot.ap()[:, gb : gb + NT], in_=o_t
                    )

    nc.finalize()
    return nc


def _fold_conv_into_w1(w_conv, W1):
    """W1e[784, 100] such that x @ W1e == conv3x3(x) @ W1 (exact linear fold)."""
    W1e = np.zeros((28, 28, H), np.float64)
    W1r = W1.astype(np.float64).reshape(26, 26, H)
    wc = w_conv.astype(np.float64)
    for di in range(3):
        for dj in range(3):
            W1e[di : di + 26, dj : dj + 26, :] += wc[di, dj] * W1r
    return W1e.reshape(784, H).astype(np.float32)


def kernel(x, w_conv, W1, b1, W2, b2, W3, b3):
    from concourse.bass_utils import run_bass_kernel_spmd

    global LAST_RESULTS

    x = np.ascontiguousarray(np.asarray(x, np.float32))
    W1e = _fold_conv_into_w1(np.asarray(w_conv), np.asarray(W1))
    # [784, 100]: rows 0..767 -> [128, KC, 100]; rows 768..783 -> [16, 100]
    w1_dev = np.ascontiguousarray(
        W1e[: KC * 128].reshape(KC, 128, H).transpose(1, 0, 2)
    )
    w1l_dev = np.ascontiguousarray(W1e[KC * 128 :])
    w23_dev = np.zeros((H, H + O), np.float32)
    w23_dev[:, 0:H] = np.asarray(W2, np.float32)
    w23_dev[:, H : H + O] = np.asarray(W3, np.float32)
    bb_dev = np.zeros((H, 3), np.float32)
    bb_dev[:, 0] = np.asarray(b1, np.float32)
    bb_dev[:, 1] = np.asarray(b2, np.float32)
    bb_dev[:O, 2] = np.asarray(b3, np.float32)

    in_maps = []
    for c in range(N_CORES):
        xs = x[c * B_LOC : (c + 1) * B_LOC]          # [B_LOC, 784]
        xT = xs.T                                     # [784, B_LOC] (view)
        # main: [N_DMA, 128, KC, NTD], element [t, p, k, n]
        #       = xT[k*128 + p, t*NTD + n]
        xmain = np.ascontiguousarray(
            xT[: KC * 128]
            .reshape(KC, 128, N_DMA, NTD)
            .transpose(2, 1, 0, 3)
        )
        xtail = np.ascontiguousarray(xT[KC * 128 :])  # [16, B_LOC]
        in_maps.append(
            {
                "xt": xmain,
                "xtl": xtail,
                "w1": w1_dev,
                "w1l": w1l_dev,
                "w23": w23_dev,
                "bb": bb_dev,
            }
        )

    if "nc" not in _COMPILED:
        _COMPILED["nc"] = _build_nc()
    nc = _COMPILED["nc"]

    res = run_bass_kernel_spmd(nc, in_maps, core_ids=list(range(N_CORES)))
    LAST_RESULTS = res

    out = np.empty((B, O), np.float32)
    for c in range(N_CORES):
        out[c * B_LOC : (c + 1) * B_LOC] = res.results[c]["ot"].T
    return out


# revision 10
# speedup vs baseline: 1.5276x; 1.2499x over previous
"""Trainium2 Bass kernel for DigitConvolutionalModel.

Reference computation (B = 32768):
    x: [B, 784] -> reshape [B, 28, 28]
    conv 3x3 valid with w_conv -> [B, 26, 26] -> [B, 676]
    h1 = relu(conv @ W1 + b1)    W1: [676, 100]
    h2 = relu(h1 @ W2 + b2)      W2: [100, 100]
    out = h2 @ W3 + b3           W3: [100, 10]

Strategy
--------
Pure data parallel: batch split 8 ways (4096 rows/core), weights replicated.
The conv is linear, so it is folded into W1 on the host:
    conv(x) @ W1 == x @ (M @ W1) = x @ W1e,  W1e: [784, 100]
removing the conv from the device entirely (exact up to fp rounding).

On-device layout is "transposed": features on SBUF partitions, batch on the
free dimension, so each layer's PSUM output feeds the next matmul directly
as the moving operand. The host pre-transposes x per core and blocks it as
[n_dma_tiles, 128, KC, NTD] (contraction split 784 = 6*128 + 16; the 16-row
tail is loaded once as a [16, B_LOC] resident tile) so every x DMA uses all
128 partitions with long contiguous runs.

Matmuls run as float32r (TF32-like single-pass mode: ~280 ns per N=512
matmul warm). The kernel is HBM-bandwidth bound streaming x (~12.8 MB/core).
"""

import numpy as np

N_CORES = 8
B = 32768
B_LOC = B // N_CORES          # 4096 rows per core
NT = 512                      # matmul moving-dim tile (fp32 max free dim)
NTD = 1024                    # batch columns per x DMA tile
N_DMA = B_LOC // NTD          # 4
HALves = NTD // NT            # 2
KC = 6                        # full 128-row contraction chunks
KT = 784 - KC * 128           # 16-row tail
H = 100                       # hidden width
O = 10                        # output width
XBUFS = 3                     # x tile double-buffer depth

_COMPILED = {}
LAST_RESULTS = None


def _build_nc():
    import concourse.mybir as mybir
    from concourse import bacc
    from concourse.tile import TileContext

    f32 = mybir.dt.float32
    fr = mybir.dt.float32r

    nc = bacc.Bacc(
        "TRN2", target_bir_lowering=False, debug=False, num_devices=N_CORES
    )
    xt = nc.dram_tensor("xt", [N_DMA, 128, KC, NTD], fr, kind="ExternalInput")
    xtl = nc.dram_tensor("xtl", [KT, B_LOC], fr, kind="ExternalInput")
    w1 = nc.dram_tensor("w1", [128, KC, H], fr, kind="ExternalInput")
    w1l = nc.dram_tensor("w1l", [KT, H], fr, kind="ExternalInput")
    # packed [100, 110]: W2 | W3
    w23 = nc.dram_tensor("w23", [H, H + O], fr, kind="ExternalInput")
    # packed [100, 3]: b1 | b2 | b3 (b3 on partitions 0..9)
    bb = nc.dram_tensor("bb", [H, 3], f32, kind="ExternalInput")
    ot = nc.dram_tensor("ot", [O, B_LOC], f32, kind="ExternalOutput")

    relu = mybir.ActivationFunctionType.Relu
    add = mybir.AluOpType.add
    amax = mybir.AluOpType.max

    with TileContext(nc) as tc:
        with (
            tc.tile_pool(name="wpool", bufs=1) as wpool,
            tc.tile_pool(name="xpool", bufs=XBUFS) as xpool,
            tc.tile_pool(name="hpool", bufs=3) as hpool,
            tc.tile_pool(name="opool", bufs=3) as opool,
            tc.tile_pool(name="ppool", bufs=2, space="PSUM") as ppool,
        ):
            # x streams on the sync HWDGE ring in per-chunk DMAs (512 KB
            # each) so the first matmul only waits for the first chunk;
            # weights go on the scalar HWDGE ring, ordered by first use.
            w1_t = wpool.tile([128, KC, H], fr)
            nc.scalar.dma_start(out=w1_t, in_=w1.ap())
            w1l_t = wpool.tile([KT, H], fr)
            nc.scalar.dma_start(out=w1l_t, in_=w1l.ap())
            xl_t = wpool.tile([KT, B_LOC], fr)
            nc.scalar.dma_start(out=xl_t, in_=xtl.ap())
            w23_t = wpool.tile([H, H + O], fr)
            nc.scalar.dma_start(out=w23_t, in_=w23.ap())
            bb_t = wpool.tile([H, 3], f32)
            nc.scalar.dma_start(out=bb_t, in_=bb.ap())

            w2_t = w23_t[:, 0:H]
            w3_t = w23_t[:, H : H + O]
            b1_t = bb_t[:, 0:1]
            b2_t = bb_t[:, 1:2]
            b3_t = bb_t[:O, 2:3]

            for t in range(N_DMA):
                x_t = xpool.tile([128, KC, NTD], fr, tag="xt")
                for c in range(KC):
                    nc.sync.dma_start(
                        out=x_t[:, c, :], in_=xt.ap()[t, :, c, :]
                    )
                ps1h = []
                for hf in range(HALves):
                    ps1 = ppool.tile([128, NT], f32, tag="ps1", bufs=3)
                    ps1h.append(ps1)
                # interleave halves per chunk: each arriving chunk feeds
                # both halves' accumulating matmuls immediately
                for c in range(KC):
                    for hf in range(HALves):
                        n0 = hf * NT
                        nc.tensor.matmul(
                            ps1h[hf][:H, :],
                            lhsT=w1_t[:, c, :],
                            rhs=x_t[:, c, n0 : n0 + NT],
                            start=(c == 0),
                            stop=False,
                        )
                for hf in range(HALves):
                    n0 = hf * NT
                    gb = t * NTD + n0  # global batch offset within core
                    ps1 = ps1h[hf]
                    nc.tensor.matmul(
                        ps1[:H, :],
                        lhsT=w1l_t,
                        rhs=xl_t[:, gb : gb + NT],
                        start=False,
                        stop=True,
                    )
                    h1 = hpool.tile([H, NT], fr, tag="h1")
                    nc.scalar.activation(h1, ps1[:H, :], relu, bias=b1_t)

                    ps2 = ppool.tile([128, NT], f32, tag="ps2", bufs=2)
                    nc.tensor.matmul(
                        ps2[:H, :], lhsT=w2_t, rhs=h1, start=True, stop=True
                    )
                    h2 = hpool.tile([H, NT], fr, tag="h2")
                    nc.vector.tensor_scalar(h2, ps2[:H, :], b2_t, 0.0, add, amax)

                    ps3 = ppool.tile([128, NT], f32, tag="ps3", bufs=2)
                    nc.tensor.matmul(
                        ps3[:O, :], lhsT=w3_t, rhs=h2, start=True, stop=True
                    )
                    o_t = opool.tile([O, NT], f32)
                    nc.vector.tensor_scalar_add(o_t, ps3[:O, :], b3_t)
                    nc.scalar.dma_start(
                        out=ot.ap()[:, gb : gb + NT], in_=o_t
                    )

    nc.finalize()
    return nc


def _fold_conv_into_w1(w_conv, W1):
    """W1e[784, 100] such that x @ W1e == conv3x3(x) @ W1 (exact linear fold)."""
    W1e = np.zeros((28, 28, H), np.float64)
    W1r = W1.astype(np.float64).reshape(26, 26, H)
    wc = w_conv.astype(np.float64)
    for di in range(3):
        for dj in range(3):
            W1e[di : di + 26, dj : dj + 26, :] += wc[di, dj] * W1r
    return W1e.reshape(784, H).astype(np.float32)


def kernel(x, w_conv, W1, b1, W2, b2, W3, b3):
    from concourse.bass_utils import run_bass_kernel_spmd

    global LAST_RESULTS

    x = np.ascontiguousarray(np.asarray(x, np.float32))
    W1e = _fold_conv_into_w1(np.asarray(w_conv), np.asarray(W1))
    # [784, 100]: rows 0..767 -> [128, KC, 100]; rows 768..783 -> [16, 100]
    w1_dev = np.ascontiguousarray(
        W1e[: KC * 128].reshape(KC, 128, H).transpose(1, 0, 2)
    )
    w1l_dev = np.ascontiguousarray(W1e[KC * 128 :])
    w23_dev = np.zeros((H, H + O), np.float32)
    w23_dev[:, 0:H] = np.asarray(W2, np.float32)
    w23_dev[:, H : H + O] = np.asarray(W3, np.float32)
    bb_dev = np.zeros((H, 3), np.float32)
    bb_dev[:, 0] = np.asarray(b1, np.float32)
    bb_dev[:, 1] = np.asarray(b2, np.float32)
    bb_dev[:O, 2] = np.asarray(b3, np.float32)

    in_maps = []
    for c in range(N_CORES):
        xs = x[c * B_LOC : (c + 1) * B_LOC]          # [B_LOC, 784]
        xT = xs.T                                     # [784, B_LOC] (view)
        # main: [N_DMA, 128, KC, NTD], element [t, p, k, n]
        #       = xT[k*128 + p, t*NTD + n]
        xmain = np.ascontiguousarray(
            xT[: KC * 128]
            .reshape(KC, 128, N_DMA, NTD)
            .transpose(2, 1, 0, 3)
        )
        xtail = np.ascontiguousarray(xT[KC * 128 :])  # [16, B_LOC]
        in_maps.append(
            {
                "xt": xmain,
                "xtl": xtail,
                "w1": w1_dev,
                "w1l": w1l_dev,
                "w23": w23_dev,
                "bb": bb_dev,
            }
        )

    if "nc" not in _COMPILED:
        _COMPILED["nc"] = _build_nc()
    nc = _COMPILED["nc"]

    res = run_bass_kernel_spmd(nc, in_maps, core_ids=list(range(N_CORES)))
    LAST_RESULTS = res

    out = np.empty((B, O), np.float32)
    for c in range(N_CORES):
        out[c * B_LOC : (c + 1) * B_LOC] = res.results[c]["ot"].T
    return out
